# revision 41
# baseline (speedup 1.0000x reference)
"""AugmentationLayer Trainium2 kernel.

Data-parallel over batch: 8 cores x 8 batch elements each.

The gumbel-softmax straight-through `selection` is exactly one-hot in the
forward pass (off entries are (0+v)-v == 0, the selected entry (1+s)-s),
so each batch element needs only its selected transform:

    out[b] = sv[b] * T_{k*}(x[b], intensity[b])

All transforms collapse into one parametric formula per batch element

    out = A*x + B + C*tanh(D*x) + E*sin(x) + F*flip(x)

with per-batch scalars from the on-device routing MLPs.  flip(x) comes
from an anti-identity fp32 matmul (partition reversal; products are
exactly 1.0*x) plus a negative-stride PSUM read (sub-block reversal) in
the ScalarE pass that applies F and B.

sin needs range reduction (the ScalarE Sin table is only accurate for
|arg| <~ pi, x reaches ~5.6): ki = rne(x/2pi + 2) on ScalarE (hardware
f32->i32 conversion is round-to-nearest-even), w = x - 2pi*ki on VectorE
(in-place over ki), sin(x) = Sin(w + 4pi), |w + 4pi| <= pi.

Engine-balance notes (from neuron-profile traces):
 - ScalarE activation-table reloads cost ~1.3us each; passes are grouped
   by table set (Copy/Identity/Sin in trig_and_small, Tanh in
   exp_and_others) and pinned in order with add_dep_helper.
 - ki/wv/sn have batch-independent scalars, so they run as 2 chunked
   instructions over 4 batch rows each (less per-instruction overhead).
"""

import numpy as np

B_FULL, L, C = 64, 2048, 64
K, F, H = 8, 24, 64
D_IN = K + F  # 32
NCORES = 8
B = B_FULL // NCORES  # 8
P = 128
LS = L // P  # 16
FW = LS * C  # 1024
NCHUNK = 2
CB = B // NCHUNK  # batches per chunk

# column layout of the host-packed routing-input tensor "wall" [H, WCOLS]
WC = {
    "pW1": (0, 64), "pW2": (64, 128), "pW3": (128, 136),
    "iW1": (136, 200), "iW2": (200, 264), "iW3": (264, 272),
    "pb1": (272, 273), "pb2": (273, 274), "ib1": (274, 275), "ib2": (275, 276),
    "pb3": (276, 284), "ib3": (284, 292), "iota": (292, 300),
    "ltb": (300, 301), "gnt": (301, 309), "mlpT": (309, 317),
}
WCOLS = 320

_CACHE = {}


def _pack_wall(pp, ft, gn, logt, w):
    wall = np.zeros((64, WCOLS), dtype=np.float32)
    wall[0:32, 0:64] = w["pW1"]
    wall[:, 64:128] = w["pW2"]
    wall[:, 128:136] = w["pW3"]
    wall[0:32, 136:200] = w["iW1"]
    wall[:, 200:264] = w["iW2"]
    wall[:, 264:272] = w["iW3"]
    wall[:, 272] = w["pb1"]
    wall[:, 273] = w["pb2"]
    wall[:, 274] = w["ib1"]
    wall[:, 275] = w["ib2"]
    wall[0:8, 276:284] = w["pb3"][None, :]
    wall[0:8, 284:292] = w["ib3"][None, :]
    wall[0:8, 292:300] = np.arange(8, dtype=np.float32)[None, :]
    wall[0:8, 300] = float(logt)
    wall[0:8, 301:309] = gn
    wall[0:32, 309:317] = np.concatenate([pp, ft], axis=1).T
    return wall


def _build_nc():
    import concourse.bass as bass
    import concourse.bacc as bacc
    import concourse.mybir as mybir
    import concourse.tile as tile
    from concourse.tile_rust import add_dep_helper

    f32 = mybir.dt.float32
    i32 = mybir.dt.int32
    AF = mybir.ActivationFunctionType
    OP = mybir.AluOpType
    AX = mybir.AxisListType

    nc = bacc.Bacc("TRN2", target_bir_lowering=False)

    xs = nc.dram_tensor("xs", [B, L, C], f32, kind="ExternalInput")
    # All routing inputs host-packed into one [H, WCOLS] tensor -> one DMA.
    wall_t = nc.dram_tensor("wall", [H, WCOLS], f32, kind="ExternalInput")

    ys = nc.dram_tensor("ys", [B, L, C], f32, kind="ExternalOutput")
    oprob = nc.dram_tensor("oprob", [B, K], f32, kind="ExternalOutput")
    oint = nc.dram_tensor("oint", [B, K], f32, kind="ExternalOutput")
    oidx = nc.dram_tensor("oidx", [B], i32, kind="ExternalOutput")

    scr = nc.dram_tensor("scr", [B * K], f32, kind="Internal")

    rmat = np.zeros((P, P), dtype=np.float32)
    rmat[np.arange(P), P - 1 - np.arange(P)] = 1.0
    c_rmat = nc.inline_tensor(rmat, "c_rmat")

    pi = float(np.pi)

    with tile.TileContext(nc) as tc:
        with (
            tc.tile_pool(name="sing", bufs=1) as sing,
            tc.tile_pool(name="rpool", bufs=2) as rpool,
            tc.tile_pool(name="spool", bufs=2) as spool,
            tc.tile_pool(name="psum", bufs=1, space="PSUM") as psum,
            tc.tile_pool(name="fpsum", bufs=2, space="PSUM") as fpsum,
            tc.tile_pool(name="big", bufs=1) as big,
            tc.tile_pool(name="opool", bufs=2) as opool,
        ):
            # ---------- big persistent tiles ----------
            xball = big.tile([P, B * FW], f32, name="xball")
            kiall = big.tile([P, B * FW], i32, name="kiall")
            snall = big.tile([P, B * FW], f32, name="snall")
            thall = big.tile([P, B * FW], f32, name="thall")
            acall = big.tile([P, B * FW], f32, name="acall")

            def bsl(t, b, n=1):
                return t[:, b * FW : (b + n) * FW]

            # ---------- routing inputs: ONE packed DMA, issued first ----------
            wall = sing.tile([H, WCOLS], f32)
            nc.sync.dma_start(out=wall, in_=wall_t[:, :])
            rsb = sing.tile([P, P], f32)
            nc.sync.dma_start(out=rsb, in_=c_rmat[:, :])

            def wsl(nm, rows=H):
                a, bb = WC[nm]
                return wall[0:rows, a:bb]

            wsb = {
                "pW1": wsl("pW1", D_IN), "pW2": wsl("pW2"), "pW3": wsl("pW3"),
                "iW1": wsl("iW1", D_IN), "iW2": wsl("iW2"), "iW3": wsl("iW3"),
            }
            bsb = {
                "pb1": wsl("pb1"), "pb2": wsl("pb2"),
                "ib1": wsl("ib1"), "ib2": wsl("ib2"),
                "pb3": wsl("pb3", B), "ib3": wsl("ib3", B),
            }
            mlpT = wsl("mlpT", D_IN)
            gnt = wsl("gnt", B)
            iotabc = wsl("iota", B)
            ltb = wsl("ltb", B)
            fourpi = sing.tile([P, 1], f32)
            nc.vector.memset(fourpi, 4.0 * pi)

            # ---------- x loads ----------
            for b in range(B):
                nc.sync.dma_start(
                    out=bsl(xball, b),
                    in_=bass.AP(tensor=xs, offset=b * L * C,
                                ap=[[LS * C, P], [1, FW]]),
                )

            # ---------- ki chunks (trig table Copy; only need x) ----------
            ki_insts = []
            for ch in range(NCHUNK):
                inst = nc.scalar.activation(
                    bsl(kiall, ch * CB, CB), bsl(xball, ch * CB, CB),
                    AF.Copy, bias=2.0, scale=1.0 / (2.0 * pi),
                )
                ki_insts.append(inst)

            # ---------- routing ----------
            def mlp(w1, b1, w2, b2, w3, b3bc):
                ph1 = psum.tile([H, B], f32)
                nc.tensor.matmul(ph1, wsb[w1], mlpT, start=True, stop=True)
                h1 = rpool.tile([H, B], f32)
                nc.scalar.activation(h1, ph1, AF.Relu, bias=bsb[b1], scale=1.0)
                ph2 = psum.tile([H, B], f32)
                nc.tensor.matmul(ph2, wsb[w2], h1, start=True, stop=True)
                h2 = rpool.tile([H, B], f32)
                nc.scalar.activation(h2, ph2, AF.Relu, bias=bsb[b2], scale=1.0)
                pl = psum.tile([B, K], f32)
                nc.tensor.matmul(pl, h2, wsb[w3], start=True, stop=True)
                lg = rpool.tile([B, K], f32)
                nc.vector.tensor_tensor(lg, pl, bsb[b3bc], op=OP.add)
                return lg

            logits = mlp("pW1", "pb1", "pW2", "pb2", "pW3", "pb3")
            ilogit = mlp("iW1", "ib1", "iW2", "ib2", "iW3", "ib3")

            ilexp = rpool.tile([B, K], f32)
            nc.scalar.activation(ilexp, ilogit, AF.Exp, bias=0.0, scale=1.0)
            inten = sing.tile([B, K], f32)
            nc.scalar.activation(inten, ilexp, AF.Ln, bias=1.0, scale=1.0)
            nc.sync.dma_start(out=oint[:, :], in_=inten)

            mx = spool.tile([B, 1], f32)
            nc.vector.reduce_max(mx, logits, axis=AX.X)
            negmx = spool.tile([B, 1], f32)
            nc.vector.tensor_scalar_mul(negmx, mx, -1.0)
            ex = rpool.tile([B, K], f32)
            nc.scalar.activation(ex, logits, AF.Exp, bias=negmx, scale=1.0)
            sume = spool.tile([B, 1], f32)
            nc.vector.reduce_sum(sume, ex, axis=AX.X)
            rse = spool.tile([B, 1], f32)
            nc.vector.reciprocal(rse, sume)
            prob = rpool.tile([B, K], f32)
            nc.vector.tensor_scalar_mul(prob, ex, rse)
            nc.sync.dma_start(out=oprob[:, :], in_=prob)

            yp = sing.tile([B, K], f32)
            nc.vector.tensor_tensor(yp, logits, gnt, op=OP.add)
            ymp = spool.tile([B, 1], f32)
            nc.vector.reduce_max(ymp, yp, axis=AX.X)
            eq = rpool.tile([B, K], f32)
            nc.vector.tensor_scalar(eq, yp, ymp, None, op0=OP.is_equal)
            iom = sing.tile([B, K], f32)
            nc.vector.tensor_scalar(iom, iotabc, 256.0, None, op0=OP.subtract)
            cnd = rpool.tile([B, K], f32)
            nc.vector.tensor_tensor(cnd, eq, iom, op=OP.mult)
            cnd2 = rpool.tile([B, K], f32)
            nc.vector.tensor_scalar(cnd2, cnd, 256.0, None, op0=OP.add)
            idxf = spool.tile([B, 1], f32)
            nc.vector.tensor_reduce(idxf, cnd2, axis=AX.X, op=OP.min)
            onehot = sing.tile([B, K], f32)
            nc.vector.tensor_scalar(onehot, iotabc, idxf, None, op0=OP.is_equal)

            taub = spool.tile([B, 1], f32)
            nc.scalar.activation(taub, ltb, AF.Exp, bias=0.0, scale=1.0)
            tauc = spool.tile([B, 1], f32)
            nc.vector.tensor_scalar(tauc, taub, 0.01, 10.0, op0=OP.max, op1=OP.min)
            rtau = spool.tile([B, 1], f32)
            nc.vector.reciprocal(rtau, tauc)
            yv = rpool.tile([B, K], f32)
            nc.vector.tensor_scalar_mul(yv, yp, rtau)
            ym = spool.tile([B, 1], f32)
            nc.vector.tensor_scalar_mul(ym, ymp, rtau)
            negym = spool.tile([B, 1], f32)
            nc.vector.tensor_scalar_mul(negym, ym, -1.0)
            ey = rpool.tile([B, K], f32)
            nc.scalar.activation(ey, yv, AF.Exp, bias=negym, scale=1.0)
            sum2 = spool.tile([B, 1], f32)
            nc.vector.reduce_sum(sum2, ey, axis=AX.X)
            sva = spool.tile([B, 1], f32)
            nc.vector.reciprocal(sva, sum2)
            sv = spool.tile([B, 1], f32)
            nc.vector.tensor_scalar(sv, sva, 1.0, sva, op0=OP.add, op1=OP.subtract)
            sel = sing.tile([B, K], f32)
            nc.vector.tensor_scalar_mul(sel, onehot, sv)

            idx32 = spool.tile([B, 1], i32)
            nc.vector.tensor_copy(idx32, idxf)
            nc.sync.dma_start(
                out=bass.AP(tensor=oidx, offset=0, ap=[[1, B], [1, 1]]), in_=idx32
            )

            # coef columns: 0:A 1:B 2:C 3:D 4:E 6:F
            coef = sing.tile([B, K], f32)
            nc.vector.memset(coef, 0.0)

            def tcol(j):
                return inten[:, j : j + 1]

            def scol(j):
                return sel[:, j : j + 1]

            def ocol(j):
                return onehot[:, j : j + 1]

            tmp = {n: spool.tile([B, 1], f32, name=f"tmp_{n}") for n in
                   ("u1", "u3", "u5", "u4", "a1", "a3", "a5",
                    "t0", "t1", "t2", "t3")}
            nc.vector.tensor_scalar(tmp["u1"], tcol(1), 1.0, None, op0=OP.add)
            nc.vector.tensor_tensor(tmp["a1"], scol(1), tmp["u1"], op=OP.mult)
            nc.vector.tensor_scalar(
                tmp["u3"], tcol(3), -1.0, 1.0, op0=OP.mult, op1=OP.add
            )
            nc.vector.tensor_tensor(tmp["a3"], scol(3), tmp["u3"], op=OP.mult)
            nc.scalar.activation(tmp["u5"], tcol(5), AF.Exp, bias=0.0, scale=-1.0)
            nc.vector.tensor_tensor(tmp["a5"], scol(5), tmp["u5"], op=OP.mult)
            # tree-shaped sum to cut the serial latency of the coef chain
            nc.vector.tensor_tensor(tmp["t0"], scol(0), scol(2), op=OP.add)
            nc.vector.tensor_tensor(tmp["t1"], scol(6), tmp["a1"], op=OP.add)
            nc.vector.tensor_tensor(tmp["t2"], tmp["a3"], tmp["a5"], op=OP.add)
            nc.vector.tensor_tensor(tmp["t3"], tmp["t0"], tmp["t1"], op=OP.add)
            nc.vector.tensor_tensor(coef[:, 0:1], tmp["t3"], tmp["t2"], op=OP.add)
            nc.vector.tensor_tensor(coef[:, 1:2], scol(2), tcol(2), op=OP.mult)
            nc.vector.tensor_copy(coef[:, 2:3], scol(4))
            nc.vector.tensor_scalar(tmp["u4"], tcol(4), 1.0, None, op0=OP.add)
            nc.vector.tensor_tensor(coef[:, 3:4], ocol(4), tmp["u4"], op=OP.mult)
            nc.vector.tensor_tensor(coef[:, 4:5], scol(6), tcol(6), op=OP.mult)
            nc.vector.tensor_copy(coef[:, 6:7], scol(7))

            nc.sync.dma_start(
                out=bass.AP(tensor=scr, offset=0, ap=[[K, B], [1, K]]), in_=coef
            )
            coefb = sing.tile([P, B * K], f32)
            nc.sync.dma_start(
                out=coefb, in_=bass.AP(tensor=scr, offset=0, ap=[[0, P], [1, B * K]])
            )

            def col(b, j):
                return coefb[:, b * K + j : b * K + j + 1]

            # ---------- flip matmuls + early psum->SBUF copy ----------
            # acall[b] = flip(x[b]) (no coefficient dependency, runs early);
            # F and B are applied later by an in-place DVE tensor_scalar.
            half = FW // 2
            ac_insts = []
            for b in range(B):
                pf = fpsum.tile([P, FW], f32, name="pf")
                nc.tensor.matmul(
                    pf[:, 0:half], rsb, bsl(xball, b)[:, 0:half],
                    start=True, stop=True,
                )
                nc.tensor.matmul(
                    pf[:, half:FW], rsb, bsl(xball, b)[:, half:FW],
                    start=True, stop=True,
                )
                pf_rev = pf.rearrange("p (q c) -> p q c", c=C)[:, ::-1, :]
                inst = nc.scalar.activation(
                    bsl(acall, b).rearrange("p (q c) -> p q c", c=C), pf_rev,
                    AF.Identity, bias=0.0, scale=1.0,
                )
                ac_insts.append(inst)

            # ---------- tanh passes (exp table), pinned after acc0 ----------
            th_insts = []
            for b in range(B):
                inst = nc.scalar.activation(
                    bsl(thall, b), bsl(xball, b), AF.Tanh,
                    bias=0.0, scale=col(b, 3),
                )
                th_insts.append(inst)

            # ---------- wv (in-place over ki) + sin chunks (trig table) ----------
            wvall = kiall.bitcast(f32)
            wv_insts, sn_insts = [], []
            for ch in range(NCHUNK):
                s = slice(ch * CB * FW, (ch + 1) * CB * FW)
                inst = nc.vector.scalar_tensor_tensor(
                    wvall[:, s], kiall[:, s], -2.0 * pi, xball[:, s],
                    op0=OP.mult, op1=OP.add,
                )
                wv_insts.append(inst)
                inst = nc.scalar.activation(
                    snall[:, s], wvall[:, s], AF.Sin, bias=fourpi, scale=1.0
                )
                sn_insts.append(inst)

            # pin ScalarE ordering to avoid activation-table thrash:
            # [Copy,Identity](trig) -> [Tanh](exp) -> [Sin](trig)
            try:
                add_dep_helper(ac_insts[-1].inst, th_insts[0].inst, False,
                               "group ACT tables: tanh after acc0")
                add_dep_helper(th_insts[-1].inst, sn_insts[0].inst, False,
                               "group ACT tables: sin after tanh")
            except Exception:
                try:
                    add_dep_helper(ac_insts[-1], th_insts[0], False,
                                   "group ACT tables: tanh after acc0")
                    add_dep_helper(th_insts[-1], sn_insts[0], False,
                                   "group ACT tables: sin after tanh")
                except Exception:
                    pass

            # ---------- combine + store ----------
            # acall[b] <- flip(x)*F + B (in place), then
            # out = ((th*C + acc0) + sn*E) + x*A
            for b in range(B):
                of = opool.tile([P, FW], f32, name="of")
                nc.vector.tensor_scalar(
                    of, bsl(acall, b), col(b, 6), col(b, 1),
                    op0=OP.mult, op1=OP.add,
                )
                o2 = opool.tile([P, FW], f32, name="o2")
                nc.vector.scalar_tensor_tensor(
                    o2, bsl(thall, b), col(b, 2), of, op0=OP.mult, op1=OP.add
                )
                o3 = opool.tile([P, FW], f32, name="o3")
                nc.vector.scalar_tensor_tensor(
                    o3, bsl(snall, b), col(b, 4), o2, op0=OP.mult, op1=OP.add
                )
                o4 = opool.tile([P, FW], f32, name="of")
                nc.vector.scalar_tensor_tensor(
                    o4, bsl(xball, b), col(b, 0), o3, op0=OP.mult, op1=OP.add
                )
                dst = bass.AP(
                    tensor=ys, offset=b * L * C, ap=[[LS * C, P], [C, LS], [1, C]]
                )
                nc.sync.dma_start(out=dst, in_=o4)

    nc.compile()
    return nc


def _get_nc():
    if "nc" not in _CACHE:
        _CACHE["nc"] = _build_nc()
    return _CACHE["nc"]


def _shard_inputs(inputs):
    f = np.ascontiguousarray
    x = np.asarray(inputs["x"], dtype=np.float32)
    pp = np.asarray(inputs["prev_prob"], dtype=np.float32)
    ft = np.asarray(inputs["features"], dtype=np.float32)
    gn = np.asarray(inputs["gumbel_noise"], dtype=np.float32)
    logt = float(np.asarray(inputs["log_temperature"], dtype=np.float32).reshape(()))
    w = {nm: np.asarray(inputs[nm], dtype=np.float32)
         for nm in ("pW1", "pb1", "pW2", "pb2", "pW3", "pb3",
                    "iW1", "ib1", "iW2", "ib2", "iW3", "ib3")}
    in_maps = []
    for c in range(NCORES):
        sl = slice(c * B, (c + 1) * B)
        m = {
            "xs": f(x[sl]),
            "wall": _pack_wall(pp[sl], ft[sl], gn[sl], logt, w),
        }
        in_maps.append(m)
    return in_maps


def run_spmd(inputs, **kwargs):
    from concourse.bass_utils import run_bass_kernel_spmd

    nc = _get_nc()
    in_maps = _shard_inputs(inputs)
    res = run_bass_kernel_spmd(nc, in_maps, core_ids=list(range(NCORES)), **kwargs)
    outs = res.results
    x_aug = np.concatenate([o["ys"] for o in outs], axis=0)
    prob = np.concatenate([o["oprob"] for o in outs], axis=0)
    inten = np.concatenate([o["oint"] for o in outs], axis=0)
    idx = np.concatenate([o["oidx"] for o in outs], axis=0).astype(np.int32)
    return (x_aug, prob, inten, idx), res


def kernel(**inputs):
    out, _ = run_spmd(inputs)
    return out


# revision 45
# speedup vs baseline: 1.2403x; 1.2403x over previous
"""AugmentationLayer Trainium2 kernel.

Data-parallel over batch: 8 cores x 8 batch elements each.

The gumbel-softmax straight-through `selection` is exactly one-hot in the
forward pass (off entries are (0+v)-v == 0, the selected entry (1+s)-s),
so each batch element needs only its selected transform:

    out[b] = sv[b] * T_{k*}(x[b], intensity[b])

All transforms collapse into one parametric formula per batch element

    out = A*x + B + C*tanh(D*x) + E*sin(x) + F*flip(x)

with per-batch scalars from the on-device routing MLPs.  flip(x) comes
from an anti-identity fp32 matmul (partition reversal; products are
exactly 1.0*x) plus a negative-stride PSUM read (sub-block reversal) in
the ScalarE pass that applies F and B.

sin needs range reduction (the ScalarE Sin table is only accurate for
|arg| <~ pi, x reaches ~5.6): ki = rne(x/2pi + 2) on ScalarE (hardware
f32->i32 conversion is round-to-nearest-even), w = x - 2pi*ki on VectorE
(in-place over ki), sin(x) = Sin(w + 4pi), |w + 4pi| <= pi.

Engine-balance notes (from neuron-profile traces):
 - ScalarE activation-table reloads cost ~1.3us each; passes are grouped
   by table set (Copy/Identity/Sin in trig_and_small, Tanh in
   exp_and_others) and pinned in order with add_dep_helper.
 - ki/wv/sn have batch-independent scalars, so they run as 2 chunked
   instructions over 4 batch rows each (less per-instruction overhead).
"""

import numpy as np

B_FULL, L, C = 64, 2048, 64
K, F, H = 8, 24, 64
D_IN = K + F  # 32
NCORES = 8
B = B_FULL // NCORES  # 8
P = 128
LS = L // P  # 16
FW = LS * C  # 1024
NCHUNK = 2
CB = B // NCHUNK  # batches per chunk

# column layout of the host-packed routing-input tensor "wall" [H, WCOLS]
WC = {
    "pW1": (0, 64), "pW2": (64, 128), "pW3": (128, 136),
    "iW1": (136, 200), "iW2": (200, 264), "iW3": (264, 272),
    "pb1": (272, 273), "pb2": (273, 274), "ib1": (274, 275), "ib2": (275, 276),
    "pb3": (276, 284), "ib3": (284, 292), "iota": (292, 300),
    "ltb": (300, 301), "gnt": (301, 309), "mlpT": (309, 317),
}
WCOLS = 320

_CACHE = {}


def _pack_wall(pp, ft, gn, logt, w):
    wall = np.zeros((64, WCOLS), dtype=np.float32)
    wall[0:32, 0:64] = w["pW1"]
    wall[:, 64:128] = w["pW2"]
    wall[:, 128:136] = w["pW3"]
    wall[0:32, 136:200] = w["iW1"]
    wall[:, 200:264] = w["iW2"]
    wall[:, 264:272] = w["iW3"]
    wall[:, 272] = w["pb1"]
    wall[:, 273] = w["pb2"]
    wall[:, 274] = w["ib1"]
    wall[:, 275] = w["ib2"]
    wall[0:8, 276:284] = w["pb3"][None, :]
    wall[0:8, 284:292] = w["ib3"][None, :]
    wall[0:8, 292:300] = np.arange(8, dtype=np.float32)[None, :]
    wall[0:8, 300] = float(logt)
    wall[0:8, 301:309] = gn
    wall[0:32, 309:317] = np.concatenate([pp, ft], axis=1).T
    return wall


def _build_nc():
    import concourse.bass as bass
    import concourse.bacc as bacc
    import concourse.mybir as mybir
    import concourse.tile as tile
    from concourse.tile_rust import add_dep_helper

    f32 = mybir.dt.float32
    i32 = mybir.dt.int32
    AF = mybir.ActivationFunctionType
    OP = mybir.AluOpType
    AX = mybir.AxisListType

    nc = bacc.Bacc("TRN2", target_bir_lowering=False)

    xs = nc.dram_tensor("xs", [B, L, C], f32, kind="ExternalInput")
    # All routing inputs host-packed into one [H, WCOLS] tensor -> one DMA.
    wall_t = nc.dram_tensor("wall", [H, WCOLS], f32, kind="ExternalInput")

    ys = nc.dram_tensor("ys", [B, L, C], f32, kind="ExternalOutput")
    oprob = nc.dram_tensor("oprob", [B, K], f32, kind="ExternalOutput")
    oint = nc.dram_tensor("oint", [B, K], f32, kind="ExternalOutput")
    oidx = nc.dram_tensor("oidx", [B], i32, kind="ExternalOutput")

    scr = nc.dram_tensor("scr", [B * K], f32, kind="Internal")

    rmat = np.zeros((P, P), dtype=np.float32)
    rmat[np.arange(P), P - 1 - np.arange(P)] = 1.0
    c_rmat = nc.inline_tensor(rmat, "c_rmat")

    pi = float(np.pi)

    with tile.TileContext(nc) as tc:
        with (
            tc.tile_pool(name="sing", bufs=1) as sing,
            tc.tile_pool(name="rpool", bufs=2) as rpool,
            tc.tile_pool(name="spool", bufs=2) as spool,
            tc.tile_pool(name="psum", bufs=1, space="PSUM") as psum,
            tc.tile_pool(name="fpsum", bufs=2, space="PSUM") as fpsum,
            tc.tile_pool(name="big", bufs=1) as big,
            tc.tile_pool(name="opool", bufs=2) as opool,
        ):
            # ---------- big persistent tiles ----------
            xball = big.tile([P, B * FW], f32, name="xball")
            kiall = big.tile([P, B * FW], i32, name="kiall")
            snall = big.tile([P, B * FW], f32, name="snall")
            thall = big.tile([P, B * FW], f32, name="thall")
            acall = big.tile([P, B * FW], f32, name="acall")

            def bsl(t, b, n=1):
                return t[:, b * FW : (b + n) * FW]

            # ---------- routing inputs: ONE packed DMA, issued first ----------
            wall = sing.tile([H, WCOLS], f32)
            nc.sync.dma_start(out=wall, in_=wall_t[:, :])
            rsb = sing.tile([P, P], f32)
            nc.sync.dma_start(out=rsb, in_=c_rmat[:, :])

            def wsl(nm, rows=H):
                a, bb = WC[nm]
                return wall[0:rows, a:bb]

            wsb = {
                "pW1": wsl("pW1", D_IN), "pW2": wsl("pW2"), "pW3": wsl("pW3"),
                "iW1": wsl("iW1", D_IN), "iW2": wsl("iW2"), "iW3": wsl("iW3"),
            }
            bsb = {
                "pb1": wsl("pb1"), "pb2": wsl("pb2"),
                "ib1": wsl("ib1"), "ib2": wsl("ib2"),
                "pb3": wsl("pb3", B), "ib3": wsl("ib3", B),
            }
            mlpT = wsl("mlpT", D_IN)
            gnt = wsl("gnt", B)
            iotabc = wsl("iota", B)
            ltb = wsl("ltb", B)
            fourpi = sing.tile([P, 1], f32)
            nc.vector.memset(fourpi, 4.0 * pi)

            # ---------- x loads ----------
            for b in range(B):
                nc.sync.dma_start(
                    out=bsl(xball, b),
                    in_=bass.AP(tensor=xs, offset=b * L * C,
                                ap=[[LS * C, P], [1, FW]]),
                )

            # ---------- ki chunks (trig table Copy; only need x) ----------
            ki_insts = []
            for ch in range(NCHUNK):
                inst = nc.scalar.activation(
                    bsl(kiall, ch * CB, CB), bsl(xball, ch * CB, CB),
                    AF.Copy, bias=2.0, scale=1.0 / (2.0 * pi),
                )
                ki_insts.append(inst)

            # ---------- routing ----------
            def mlp(w1, b1, w2, b2, w3, b3bc):
                ph1 = psum.tile([H, B], f32)
                nc.tensor.matmul(ph1, wsb[w1], mlpT, start=True, stop=True)
                h1 = rpool.tile([H, B], f32)
                nc.scalar.activation(h1, ph1, AF.Relu, bias=bsb[b1], scale=1.0)
                ph2 = psum.tile([H, B], f32)
                nc.tensor.matmul(ph2, wsb[w2], h1, start=True, stop=True)
                h2 = rpool.tile([H, B], f32)
                nc.scalar.activation(h2, ph2, AF.Relu, bias=bsb[b2], scale=1.0)
                pl = psum.tile([B, K], f32)
                nc.tensor.matmul(pl, h2, wsb[w3], start=True, stop=True)
                lg = rpool.tile([B, K], f32)
                nc.vector.tensor_tensor(lg, pl, bsb[b3bc], op=OP.add)
                return lg

            logits = mlp("pW1", "pb1", "pW2", "pb2", "pW3", "pb3")
            ilogit = mlp("iW1", "ib1", "iW2", "ib2", "iW3", "ib3")

            ilexp = rpool.tile([B, K], f32)
            nc.scalar.activation(ilexp, ilogit, AF.Exp, bias=0.0, scale=1.0)
            inten = sing.tile([B, K], f32)
            nc.scalar.activation(inten, ilexp, AF.Ln, bias=1.0, scale=1.0)
            nc.sync.dma_start(out=oint[:, :], in_=inten)

            mx = spool.tile([B, 1], f32)
            nc.vector.reduce_max(mx, logits, axis=AX.X)
            negmx = spool.tile([B, 1], f32)
            nc.vector.tensor_scalar_mul(negmx, mx, -1.0)
            ex = rpool.tile([B, K], f32)
            nc.scalar.activation(ex, logits, AF.Exp, bias=negmx, scale=1.0)
            sume = spool.tile([B, 1], f32)
            nc.vector.reduce_sum(sume, ex, axis=AX.X)
            rse = spool.tile([B, 1], f32)
            nc.vector.reciprocal(rse, sume)
            prob = rpool.tile([B, K], f32)
            nc.vector.tensor_scalar_mul(prob, ex, rse)
            nc.sync.dma_start(out=oprob[:, :], in_=prob)

            yp = sing.tile([B, K], f32)
            nc.vector.tensor_tensor(yp, logits, gnt, op=OP.add)
            ymp = spool.tile([B, 1], f32)
            nc.vector.reduce_max(ymp, yp, axis=AX.X)
            eq = rpool.tile([B, K], f32)
            nc.vector.tensor_scalar(eq, yp, ymp, None, op0=OP.is_equal)
            iom = sing.tile([B, K], f32)
            nc.vector.tensor_scalar(iom, iotabc, 256.0, None, op0=OP.subtract)
            cnd = rpool.tile([B, K], f32)
            nc.vector.tensor_tensor(cnd, eq, iom, op=OP.mult)
            cnd2 = rpool.tile([B, K], f32)
            nc.vector.tensor_scalar(cnd2, cnd, 256.0, None, op0=OP.add)
            idxf = spool.tile([B, 1], f32)
            nc.vector.tensor_reduce(idxf, cnd2, axis=AX.X, op=OP.min)
            onehot = sing.tile([B, K], f32)
            nc.vector.tensor_scalar(onehot, iotabc, idxf, None, op0=OP.is_equal)

            taub = spool.tile([B, 1], f32)
            nc.scalar.activation(taub, ltb, AF.Exp, bias=0.0, scale=1.0)
            tauc = spool.tile([B, 1], f32)
            nc.vector.tensor_scalar(tauc, taub, 0.01, 10.0, op0=OP.max, op1=OP.min)
            rtau = spool.tile([B, 1], f32)
            nc.vector.reciprocal(rtau, tauc)
            yv = rpool.tile([B, K], f32)
            nc.vector.tensor_scalar_mul(yv, yp, rtau)
            ym = spool.tile([B, 1], f32)
            nc.vector.tensor_scalar_mul(ym, ymp, rtau)
            negym = spool.tile([B, 1], f32)
            nc.vector.tensor_scalar_mul(negym, ym, -1.0)
            ey = rpool.tile([B, K], f32)
            nc.scalar.activation(ey, yv, AF.Exp, bias=negym, scale=1.0)
            sum2 = spool.tile([B, 1], f32)
            nc.vector.reduce_sum(sum2, ey, axis=AX.X)
            sva = spool.tile([B, 1], f32)
            nc.vector.reciprocal(sva, sum2)
            sv = spool.tile([B, 1], f32)
            nc.vector.tensor_scalar(sv, sva, 1.0, sva, op0=OP.add, op1=OP.subtract)
            sel = sing.tile([B, K], f32)
            nc.vector.tensor_scalar_mul(sel, onehot, sv)

            idx32 = spool.tile([B, 1], i32)
            nc.vector.tensor_copy(idx32, idxf)
            nc.sync.dma_start(
                out=bass.AP(tensor=oidx, offset=0, ap=[[1, B], [1, 1]]), in_=idx32
            )

            # coef columns: 0:A 1:B 2:C 3:D 4:E 6:F
            coef = sing.tile([B, K], f32)
            nc.vector.memset(coef, 0.0)

            def tcol(j):
                return inten[:, j : j + 1]

            def scol(j):
                return sel[:, j : j + 1]

            def ocol(j):
                return onehot[:, j : j + 1]

            tmp = {n: spool.tile([B, 1], f32, name=f"tmp_{n}") for n in
                   ("u1", "u3", "u5", "u4", "a1", "a3", "a5",
                    "t0", "t1", "t2", "t3")}
            nc.vector.tensor_scalar(tmp["u1"], tcol(1), 1.0, None, op0=OP.add)
            nc.vector.tensor_tensor(tmp["a1"], scol(1), tmp["u1"], op=OP.mult)
            nc.vector.tensor_scalar(
                tmp["u3"], tcol(3), -1.0, 1.0, op0=OP.mult, op1=OP.add
            )
            nc.vector.tensor_tensor(tmp["a3"], scol(3), tmp["u3"], op=OP.mult)
            nc.scalar.activation(tmp["u5"], tcol(5), AF.Exp, bias=0.0, scale=-1.0)
            nc.vector.tensor_tensor(tmp["a5"], scol(5), tmp["u5"], op=OP.mult)
            # tree-shaped sum to cut the serial latency of the coef chain
            nc.vector.tensor_tensor(tmp["t0"], scol(0), scol(2), op=OP.add)
            nc.vector.tensor_tensor(tmp["t1"], scol(6), tmp["a1"], op=OP.add)
            nc.vector.tensor_tensor(tmp["t2"], tmp["a3"], tmp["a5"], op=OP.add)
            nc.vector.tensor_tensor(tmp["t3"], tmp["t0"], tmp["t1"], op=OP.add)
            nc.vector.tensor_tensor(coef[:, 0:1], tmp["t3"], tmp["t2"], op=OP.add)
            nc.vector.tensor_tensor(coef[:, 1:2], scol(2), tcol(2), op=OP.mult)
            nc.vector.tensor_copy(coef[:, 2:3], scol(4))
            nc.vector.tensor_scalar(tmp["u4"], tcol(4), 1.0, None, op0=OP.add)
            nc.vector.tensor_tensor(coef[:, 3:4], ocol(4), tmp["u4"], op=OP.mult)
            nc.vector.tensor_tensor(coef[:, 4:5], scol(6), tcol(6), op=OP.mult)
            nc.vector.tensor_copy(coef[:, 6:7], scol(7))

            nc.sync.dma_start(
                out=bass.AP(tensor=scr, offset=0, ap=[[K, B], [1, K]]), in_=coef
            )
            coefb = sing.tile([P, B * K], f32)
            nc.sync.dma_start(
                out=coefb, in_=bass.AP(tensor=scr, offset=0, ap=[[0, P], [1, B * K]])
            )

            def col(b, j):
                return coefb[:, b * K + j : b * K + j + 1]

            # ---------- flip matmuls + acc0 = F*flip(x) + B (trig table) ----------
            half = FW // 2
            ac_insts = []
            for b in range(B):
                pf = fpsum.tile([P, FW], f32, name="pf")
                nc.tensor.matmul(
                    pf[:, 0:half], rsb, bsl(xball, b)[:, 0:half],
                    start=True, stop=True,
                )
                nc.tensor.matmul(
                    pf[:, half:FW], rsb, bsl(xball, b)[:, half:FW],
                    start=True, stop=True,
                )
                pf_rev = pf.rearrange("p (q c) -> p q c", c=C)[:, ::-1, :]
                inst = nc.scalar.activation(
                    bsl(acall, b).rearrange("p (q c) -> p q c", c=C), pf_rev,
                    AF.Identity, bias=col(b, 1), scale=col(b, 6),
                )
                ac_insts.append(inst)

            # ---------- tanh passes (exp table), pinned after acc0 ----------
            th_insts = []
            for b in range(B):
                inst = nc.scalar.activation(
                    bsl(thall, b), bsl(xball, b), AF.Tanh,
                    bias=0.0, scale=col(b, 3),
                )
                th_insts.append(inst)

            # ---------- wv (in-place over ki) + sin chunks (trig table) ----------
            wvall = kiall.bitcast(f32)
            wv_insts, sn_insts = [], []
            for ch in range(NCHUNK):
                s = slice(ch * CB * FW, (ch + 1) * CB * FW)
                inst = nc.vector.scalar_tensor_tensor(
                    wvall[:, s], kiall[:, s], -2.0 * pi, xball[:, s],
                    op0=OP.mult, op1=OP.add,
                )
                wv_insts.append(inst)
                inst = nc.scalar.activation(
                    snall[:, s], wvall[:, s], AF.Sin, bias=fourpi, scale=1.0
                )
                sn_insts.append(inst)

            # pin ScalarE ordering to avoid activation-table thrash:
            # [Copy,Identity](trig) -> [Tanh](exp) -> [Sin](trig)
            try:
                add_dep_helper(ac_insts[-1].inst, th_insts[0].inst, False,
                               "group ACT tables: tanh after acc0")
                add_dep_helper(th_insts[-1].inst, sn_insts[0].inst, False,
                               "group ACT tables: sin after tanh")
            except Exception:
                try:
                    add_dep_helper(ac_insts[-1], th_insts[0], False,
                                   "group ACT tables: tanh after acc0")
                    add_dep_helper(th_insts[-1], sn_insts[0], False,
                                   "group ACT tables: sin after tanh")
                except Exception:
                    pass

            # ---------- combine + store ----------
            # out = ((th*C + acc0) + sn*E) + x*A   (acc0 carries B and the flip)
            for b in range(B):
                o2 = opool.tile([P, FW], f32, name="o2")
                nc.vector.scalar_tensor_tensor(
                    o2, bsl(thall, b), col(b, 2), bsl(acall, b),
                    op0=OP.mult, op1=OP.add,
                )
                o3 = opool.tile([P, FW], f32, name="o3")
                nc.vector.scalar_tensor_tensor(
                    o3, bsl(snall, b), col(b, 4), o2, op0=OP.mult, op1=OP.add
                )
                o4 = opool.tile([P, FW], f32, name="o4")
                nc.vector.scalar_tensor_tensor(
                    o4, bsl(xball, b), col(b, 0), o3, op0=OP.mult, op1=OP.add
                )
                dst = bass.AP(
                    tensor=ys, offset=b * L * C, ap=[[LS * C, P], [C, LS], [1, C]]
                )
                nc.sync.dma_start(out=dst, in_=o4)

    nc.compile()
    return nc


def _get_nc():
    if "nc" not in _CACHE:
        _CACHE["nc"] = _build_nc()
    return _CACHE["nc"]


def _shard_inputs(inputs):
    f = np.ascontiguousarray
    x = np.asarray(inputs["x"], dtype=np.float32)
    pp = np.asarray(inputs["prev_prob"], dtype=np.float32)
    ft = np.asarray(inputs["features"], dtype=np.float32)
    gn = np.asarray(inputs["gumbel_noise"], dtype=np.float32)
    logt = float(np.asarray(inputs["log_temperature"], dtype=np.float32).reshape(()))
    w = {nm: np.asarray(inputs[nm], dtype=np.float32)
         for nm in ("pW1", "pb1", "pW2", "pb2", "pW3", "pb3",
                    "iW1", "ib1", "iW2", "ib2", "iW3", "ib3")}
    in_maps = []
    for c in range(NCORES):
        sl = slice(c * B, (c + 1) * B)
        m = {
            "xs": f(x[sl]),
            "wall": _pack_wall(pp[sl], ft[sl], gn[sl], logt, w),
        }
        in_maps.append(m)
    return in_maps


def run_spmd(inputs, **kwargs):
    from concourse.bass_utils import run_bass_kernel_spmd

    nc = _get_nc()
    in_maps = _shard_inputs(inputs)
    res = run_bass_kernel_spmd(nc, in_maps, core_ids=list(range(NCORES)), **kwargs)
    outs = res.results
    x_aug = np.concatenate([o["ys"] for o in outs], axis=0)
    prob = np.concatenate([o["oprob"] for o in outs], axis=0)
    inten = np.concatenate([o["oint"] for o in outs], axis=0)
    idx = np.concatenate([o["oidx"] for o in outs], axis=0).astype(np.int32)
    return (x_aug, prob, inten, idx), res


def kernel(**inputs):
    out, _ = run_spmd(inputs)
    return out


# revision 46
# speedup vs baseline: 1.2545x; 1.0114x over previous
"""AugmentationLayer Trainium2 kernel.

Data-parallel over batch: 8 cores x 8 batch elements each.

The gumbel-softmax straight-through `selection` is exactly one-hot in the
forward pass (off entries are (0+v)-v == 0, the selected entry (1+s)-s),
so each batch element needs only its selected transform:

    out[b] = sv[b] * T_{k*}(x[b], intensity[b])

All transforms collapse into one parametric formula per batch element

    out = A*x + B + C*tanh(D*x) + E*sin(x) + F*flip(x)

with per-batch scalars from the on-device routing MLPs.  flip(x) comes
from an anti-identity fp32 matmul (partition reversal; products are
exactly 1.0*x) plus a negative-stride PSUM read (sub-block reversal) in
the ScalarE pass that applies F and B.

sin needs range reduction (the ScalarE Sin table is only accurate for
|arg| <~ pi, x reaches ~5.6): ki = rne(x/2pi + 2) on ScalarE (hardware
f32->i32 conversion is round-to-nearest-even), w = x - 2pi*ki on VectorE
(in-place over ki), sin(x) = Sin(w + 4pi), |w + 4pi| <= pi.

Engine-balance notes (from neuron-profile traces):
 - ScalarE activation-table reloads cost ~1.3us each; passes are grouped
   by table set (Copy/Identity/Sin in trig_and_small, Tanh in
   exp_and_others) and pinned in order with add_dep_helper.
 - ki/wv/sn have batch-independent scalars, so they run as 2 chunked
   instructions over 4 batch rows each (less per-instruction overhead).
"""

import numpy as np

B_FULL, L, C = 64, 2048, 64
K, F, H = 8, 24, 64
D_IN = K + F  # 32
NCORES = 8
B = B_FULL // NCORES  # 8
P = 128
LS = L // P  # 16
FW = LS * C  # 1024
NCHUNK = 2
CB = B // NCHUNK  # batches per chunk

# column layout of the host-packed routing-input tensor "wall" [H, WCOLS]
WC = {
    "pW1": (0, 64), "pW2": (64, 128), "pW3": (128, 136),
    "iW1": (136, 200), "iW2": (200, 264), "iW3": (264, 272),
    "pb1": (272, 273), "pb2": (273, 274), "ib1": (274, 275), "ib2": (275, 276),
    "pb3": (276, 284), "ib3": (284, 292), "iota": (292, 300),
    "ltb": (300, 301), "gnt": (301, 309), "mlpT": (309, 317),
}
WCOLS = 320

_CACHE = {}


def _pack_wall(pp, ft, gn, logt, w):
    wall = np.zeros((64, WCOLS), dtype=np.float32)
    wall[0:32, 0:64] = w["pW1"]
    wall[:, 64:128] = w["pW2"]
    wall[:, 128:136] = w["pW3"]
    wall[0:32, 136:200] = w["iW1"]
    wall[:, 200:264] = w["iW2"]
    wall[:, 264:272] = w["iW3"]
    wall[:, 272] = w["pb1"]
    wall[:, 273] = w["pb2"]
    wall[:, 274] = w["ib1"]
    wall[:, 275] = w["ib2"]
    wall[0:8, 276:284] = w["pb3"][None, :]
    wall[0:8, 284:292] = w["ib3"][None, :]
    wall[0:8, 292:300] = np.arange(8, dtype=np.float32)[None, :]
    wall[0:8, 300] = float(logt)
    wall[0:8, 301:309] = gn
    wall[0:32, 309:317] = np.concatenate([pp, ft], axis=1).T
    return wall


def _build_nc():
    import concourse.bass as bass
    import concourse.bacc as bacc
    import concourse.mybir as mybir
    import concourse.tile as tile
    from concourse.tile_rust import add_dep_helper

    f32 = mybir.dt.float32
    i32 = mybir.dt.int32
    AF = mybir.ActivationFunctionType
    OP = mybir.AluOpType
    AX = mybir.AxisListType

    nc = bacc.Bacc("TRN2", target_bir_lowering=False)

    xs = nc.dram_tensor("xs", [B, L, C], f32, kind="ExternalInput")
    # All routing inputs host-packed into one [H, WCOLS] tensor -> one DMA.
    wall_t = nc.dram_tensor("wall", [H, WCOLS], f32, kind="ExternalInput")

    ys = nc.dram_tensor("ys", [B, L, C], f32, kind="ExternalOutput")
    oprob = nc.dram_tensor("oprob", [B, K], f32, kind="ExternalOutput")
    oint = nc.dram_tensor("oint", [B, K], f32, kind="ExternalOutput")
    oidx = nc.dram_tensor("oidx", [B], i32, kind="ExternalOutput")

    scr = nc.dram_tensor("scr", [B * K], f32, kind="Internal")

    rmat = np.zeros((P, P), dtype=np.float32)
    rmat[np.arange(P), P - 1 - np.arange(P)] = 1.0
    c_rmat = nc.inline_tensor(rmat, "c_rmat")

    pi = float(np.pi)

    with tile.TileContext(nc) as tc:
        with (
            tc.tile_pool(name="sing", bufs=1) as sing,
            tc.tile_pool(name="rpool", bufs=2) as rpool,
            tc.tile_pool(name="spool", bufs=2) as spool,
            tc.tile_pool(name="psum", bufs=1, space="PSUM") as psum,
            tc.tile_pool(name="fpsum", bufs=2, space="PSUM") as fpsum,
            tc.tile_pool(name="big", bufs=1) as big,
            tc.tile_pool(name="opool", bufs=2) as opool,
        ):
            # ---------- big persistent tiles ----------
            xball = big.tile([P, B * FW], f32, name="xball")
            kiall = big.tile([P, B * FW], i32, name="kiall")
            snall = big.tile([P, B * FW], f32, name="snall")
            thall = big.tile([P, B * FW], f32, name="thall")
            acall = big.tile([P, B * FW], f32, name="acall")

            def bsl(t, b, n=1):
                return t[:, b * FW : (b + n) * FW]

            # ---------- routing inputs: ONE packed DMA, issued first ----------
            wall = sing.tile([H, WCOLS], f32)
            nc.sync.dma_start(out=wall, in_=wall_t[:, :])
            rsb = sing.tile([P, P], f32)
            nc.sync.dma_start(out=rsb, in_=c_rmat[:, :])

            def wsl(nm, rows=H):
                a, bb = WC[nm]
                return wall[0:rows, a:bb]

            wsb = {
                "pW1": wsl("pW1", D_IN), "pW2": wsl("pW2"), "pW3": wsl("pW3"),
                "iW1": wsl("iW1", D_IN), "iW2": wsl("iW2"), "iW3": wsl("iW3"),
            }
            bsb = {
                "pb1": wsl("pb1"), "pb2": wsl("pb2"),
                "ib1": wsl("ib1"), "ib2": wsl("ib2"),
                "pb3": wsl("pb3", B), "ib3": wsl("ib3", B),
            }
            mlpT = wsl("mlpT", D_IN)
            gnt = wsl("gnt", B)
            iotabc = wsl("iota", B)
            ltb = wsl("ltb", B)
            fourpi = sing.tile([P, 1], f32)
            nc.vector.memset(fourpi, 4.0 * pi)

            # ---------- x loads ----------
            for b in range(B):
                nc.sync.dma_start(
                    out=bsl(xball, b),
                    in_=bass.AP(tensor=xs, offset=b * L * C,
                                ap=[[LS * C, P], [1, FW]]),
                )

            # ---------- ki chunks (only need x) ----------
            # Split across engines to balance load: ScalarE (table-set Copy)
            # and VectorE (tensor_scalar, int32 write also rounds-to-nearest).
            ki_insts = []
            for ch in range(NCHUNK):
                if ch % 2 == 0:
                    inst = nc.scalar.activation(
                        bsl(kiall, ch * CB, CB), bsl(xball, ch * CB, CB),
                        AF.Copy, bias=2.0, scale=1.0 / (2.0 * pi),
                    )
                else:
                    inst = nc.vector.tensor_scalar(
                        bsl(kiall, ch * CB, CB), bsl(xball, ch * CB, CB),
                        1.0 / (2.0 * pi), 2.0, op0=OP.mult, op1=OP.add,
                    )
                ki_insts.append(inst)

            # ---------- routing ----------
            def mlp(w1, b1, w2, b2, w3, b3bc):
                ph1 = psum.tile([H, B], f32)
                nc.tensor.matmul(ph1, wsb[w1], mlpT, start=True, stop=True)
                h1 = rpool.tile([H, B], f32)
                nc.scalar.activation(h1, ph1, AF.Relu, bias=bsb[b1], scale=1.0)
                ph2 = psum.tile([H, B], f32)
                nc.tensor.matmul(ph2, wsb[w2], h1, start=True, stop=True)
                h2 = rpool.tile([H, B], f32)
                nc.scalar.activation(h2, ph2, AF.Relu, bias=bsb[b2], scale=1.0)
                pl = psum.tile([B, K], f32)
                nc.tensor.matmul(pl, h2, wsb[w3], start=True, stop=True)
                lg = rpool.tile([B, K], f32)
                nc.vector.tensor_tensor(lg, pl, bsb[b3bc], op=OP.add)
                return lg

            logits = mlp("pW1", "pb1", "pW2", "pb2", "pW3", "pb3")
            ilogit = mlp("iW1", "ib1", "iW2", "ib2", "iW3", "ib3")

            ilexp = rpool.tile([B, K], f32)
            nc.scalar.activation(ilexp, ilogit, AF.Exp, bias=0.0, scale=1.0)
            inten = sing.tile([B, K], f32)
            nc.scalar.activation(inten, ilexp, AF.Ln, bias=1.0, scale=1.0)
            nc.sync.dma_start(out=oint[:, :], in_=inten)

            mx = spool.tile([B, 1], f32)
            nc.vector.reduce_max(mx, logits, axis=AX.X)
            negmx = spool.tile([B, 1], f32)
            nc.vector.tensor_scalar_mul(negmx, mx, -1.0)
            ex = rpool.tile([B, K], f32)
            nc.scalar.activation(ex, logits, AF.Exp, bias=negmx, scale=1.0)
            sume = spool.tile([B, 1], f32)
            nc.vector.reduce_sum(sume, ex, axis=AX.X)
            rse = spool.tile([B, 1], f32)
            nc.vector.reciprocal(rse, sume)
            prob = rpool.tile([B, K], f32)
            nc.vector.tensor_scalar_mul(prob, ex, rse)
            nc.sync.dma_start(out=oprob[:, :], in_=prob)

            yp = sing.tile([B, K], f32)
            nc.vector.tensor_tensor(yp, logits, gnt, op=OP.add)
            ymp = spool.tile([B, 1], f32)
            nc.vector.reduce_max(ymp, yp, axis=AX.X)
            eq = rpool.tile([B, K], f32)
            nc.vector.tensor_scalar(eq, yp, ymp, None, op0=OP.is_equal)
            iom = sing.tile([B, K], f32)
            nc.vector.tensor_scalar(iom, iotabc, 256.0, None, op0=OP.subtract)
            cnd = rpool.tile([B, K], f32)
            nc.vector.tensor_tensor(cnd, eq, iom, op=OP.mult)
            cnd2 = rpool.tile([B, K], f32)
            nc.vector.tensor_scalar(cnd2, cnd, 256.0, None, op0=OP.add)
            idxf = spool.tile([B, 1], f32)
            nc.vector.tensor_reduce(idxf, cnd2, axis=AX.X, op=OP.min)
            onehot = sing.tile([B, K], f32)
            nc.vector.tensor_scalar(onehot, iotabc, idxf, None, op0=OP.is_equal)

            taub = spool.tile([B, 1], f32)
            nc.scalar.activation(taub, ltb, AF.Exp, bias=0.0, scale=1.0)
            tauc = spool.tile([B, 1], f32)
            nc.vector.tensor_scalar(tauc, taub, 0.01, 10.0, op0=OP.max, op1=OP.min)
            rtau = spool.tile([B, 1], f32)
            nc.vector.reciprocal(rtau, tauc)
            yv = rpool.tile([B, K], f32)
            nc.vector.tensor_scalar_mul(yv, yp, rtau)
            ym = spool.tile([B, 1], f32)
            nc.vector.tensor_scalar_mul(ym, ymp, rtau)
            negym = spool.tile([B, 1], f32)
            nc.vector.tensor_scalar_mul(negym, ym, -1.0)
            ey = rpool.tile([B, K], f32)
            nc.scalar.activation(ey, yv, AF.Exp, bias=negym, scale=1.0)
            sum2 = spool.tile([B, 1], f32)
            nc.vector.reduce_sum(sum2, ey, axis=AX.X)
            sva = spool.tile([B, 1], f32)
            nc.vector.reciprocal(sva, sum2)
            sv = spool.tile([B, 1], f32)
            nc.vector.tensor_scalar(sv, sva, 1.0, sva, op0=OP.add, op1=OP.subtract)
            sel = sing.tile([B, K], f32)
            nc.vector.tensor_scalar_mul(sel, onehot, sv)

            idx32 = spool.tile([B, 1], i32)
            nc.vector.tensor_copy(idx32, idxf)
            nc.sync.dma_start(
                out=bass.AP(tensor=oidx, offset=0, ap=[[1, B], [1, 1]]), in_=idx32
            )

            # coef columns: 0:A 1:B 2:C 3:D 4:E 6:F
            coef = sing.tile([B, K], f32)
            nc.vector.memset(coef, 0.0)

            def tcol(j):
                return inten[:, j : j + 1]

            def scol(j):
                return sel[:, j : j + 1]

            def ocol(j):
                return onehot[:, j : j + 1]

            tmp = {n: spool.tile([B, 1], f32, name=f"tmp_{n}") for n in
                   ("u1", "u3", "u5", "u4", "a1", "a3", "a5",
                    "t0", "t1", "t2", "t3")}
            nc.vector.tensor_scalar(tmp["u1"], tcol(1), 1.0, None, op0=OP.add)
            nc.vector.tensor_tensor(tmp["a1"], scol(1), tmp["u1"], op=OP.mult)
            nc.vector.tensor_scalar(
                tmp["u3"], tcol(3), -1.0, 1.0, op0=OP.mult, op1=OP.add
            )
            nc.vector.tensor_tensor(tmp["a3"], scol(3), tmp["u3"], op=OP.mult)
            nc.scalar.activation(tmp["u5"], tcol(5), AF.Exp, bias=0.0, scale=-1.0)
            nc.vector.tensor_tensor(tmp["a5"], scol(5), tmp["u5"], op=OP.mult)
            # tree-shaped sum to cut the serial latency of the coef chain
            nc.vector.tensor_tensor(tmp["t0"], scol(0), scol(2), op=OP.add)
            nc.vector.tensor_tensor(tmp["t1"], scol(6), tmp["a1"], op=OP.add)
            nc.vector.tensor_tensor(tmp["t2"], tmp["a3"], tmp["a5"], op=OP.add)
            nc.vector.tensor_tensor(tmp["t3"], tmp["t0"], tmp["t1"], op=OP.add)
            nc.vector.tensor_tensor(coef[:, 0:1], tmp["t3"], tmp["t2"], op=OP.add)
            nc.vector.tensor_tensor(coef[:, 1:2], scol(2), tcol(2), op=OP.mult)
            nc.vector.tensor_copy(coef[:, 2:3], scol(4))
            nc.vector.tensor_scalar(tmp["u4"], tcol(4), 1.0, None, op0=OP.add)
            nc.vector.tensor_tensor(coef[:, 3:4], ocol(4), tmp["u4"], op=OP.mult)
            nc.vector.tensor_tensor(coef[:, 4:5], scol(6), tcol(6), op=OP.mult)
            nc.vector.tensor_copy(coef[:, 6:7], scol(7))

            nc.sync.dma_start(
                out=bass.AP(tensor=scr, offset=0, ap=[[K, B], [1, K]]), in_=coef
            )
            coefb = sing.tile([P, B * K], f32)
            nc.sync.dma_start(
                out=coefb, in_=bass.AP(tensor=scr, offset=0, ap=[[0, P], [1, B * K]])
            )

            def col(b, j):
                return coefb[:, b * K + j : b * K + j + 1]

            # ---------- flip matmuls + acc0 = F*flip(x) + B (trig table) ----------
            half = FW // 2
            ac_insts = []
            for b in range(B):
                pf = fpsum.tile([P, FW], f32, name="pf")
                nc.tensor.matmul(
                    pf[:, 0:half], rsb, bsl(xball, b)[:, 0:half],
                    start=True, stop=True,
                )
                nc.tensor.matmul(
                    pf[:, half:FW], rsb, bsl(xball, b)[:, half:FW],
                    start=True, stop=True,
                )
                pf_rev = pf.rearrange("p (q c) -> p q c", c=C)[:, ::-1, :]
                inst = nc.scalar.activation(
                    bsl(acall, b).rearrange("p (q c) -> p q c", c=C), pf_rev,
                    AF.Identity, bias=col(b, 1), scale=col(b, 6),
                )
                ac_insts.append(inst)

            # ---------- tanh passes (exp table), pinned after acc0 ----------
            th_insts = []
            for b in range(B):
                inst = nc.scalar.activation(
                    bsl(thall, b), bsl(xball, b), AF.Tanh,
                    bias=0.0, scale=col(b, 3),
                )
                th_insts.append(inst)

            # ---------- wv (in-place over ki) + sin chunks (trig table) ----------
            wvall = kiall.bitcast(f32)
            wv_insts, sn_insts = [], []
            for ch in range(NCHUNK):
                s = slice(ch * CB * FW, (ch + 1) * CB * FW)
                inst = nc.vector.scalar_tensor_tensor(
                    wvall[:, s], kiall[:, s], -2.0 * pi, xball[:, s],
                    op0=OP.mult, op1=OP.add,
                )
                wv_insts.append(inst)
                inst = nc.scalar.activation(
                    snall[:, s], wvall[:, s], AF.Sin, bias=fourpi, scale=1.0
                )
                sn_insts.append(inst)

            # pin ScalarE ordering to avoid activation-table thrash:
            # [Copy,Identity](trig) -> [Tanh](exp) -> [Sin](trig)
            try:
                add_dep_helper(ac_insts[-1].inst, th_insts[0].inst, False,
                               "group ACT tables: tanh after acc0")
                add_dep_helper(th_insts[-1].inst, sn_insts[0].inst, False,
                               "group ACT tables: sin after tanh")
            except Exception:
                try:
                    add_dep_helper(ac_insts[-1], th_insts[0], False,
                                   "group ACT tables: tanh after acc0")
                    add_dep_helper(th_insts[-1], sn_insts[0], False,
                                   "group ACT tables: sin after tanh")
                except Exception:
                    pass

            # ---------- combine + store ----------
            # out = ((th*C + acc0) + sn*E) + x*A   (acc0 carries B and the flip)
            for b in range(B):
                o2 = opool.tile([P, FW], f32, name="o2")
                nc.vector.scalar_tensor_tensor(
                    o2, bsl(thall, b), col(b, 2), bsl(acall, b),
                    op0=OP.mult, op1=OP.add,
                )
                o3 = opool.tile([P, FW], f32, name="o3")
                nc.vector.scalar_tensor_tensor(
                    o3, bsl(snall, b), col(b, 4), o2, op0=OP.mult, op1=OP.add
                )
                o4 = opool.tile([P, FW], f32, name="o4")
                nc.vector.scalar_tensor_tensor(
                    o4, bsl(xball, b), col(b, 0), o3, op0=OP.mult, op1=OP.add
                )
                dst = bass.AP(
                    tensor=ys, offset=b * L * C, ap=[[LS * C, P], [C, LS], [1, C]]
                )
                nc.sync.dma_start(out=dst, in_=o4)

    nc.compile()
    return nc


def _get_nc():
    if "nc" not in _CACHE:
        _CACHE["nc"] = _build_nc()
    return _CACHE["nc"]


def _shard_inputs(inputs):
    f = np.ascontiguousarray
    x = np.asarray(inputs["x"], dtype=np.float32)
    pp = np.asarray(inputs["prev_prob"], dtype=np.float32)
    ft = np.asarray(inputs["features"], dtype=np.float32)
    gn = np.asarray(inputs["gumbel_noise"], dtype=np.float32)
    logt = float(np.asarray(inputs["log_temperature"], dtype=np.float32).reshape(()))
    w = {nm: np.asarray(inputs[nm], dtype=np.float32)
         for nm in ("pW1", "pb1", "pW2", "pb2", "pW3", "pb3",
                    "iW1", "ib1", "iW2", "ib2", "iW3", "ib3")}
    in_maps = []
    for c in range(NCORES):
        sl = slice(c * B, (c + 1) * B)
        m = {
            "xs": f(x[sl]),
            "wall": _pack_wall(pp[sl], ft[sl], gn[sl], logt, w),
        }
        in_maps.append(m)
    return in_maps


def run_spmd(inputs, **kwargs):
    from concourse.bass_utils import run_bass_kernel_spmd

    nc = _get_nc()
    in_maps = _shard_inputs(inputs)
    res = run_bass_kernel_spmd(nc, in_maps, core_ids=list(range(NCORES)), **kwargs)
    outs = res.results
    x_aug = np.concatenate([o["ys"] for o in outs], axis=0)
    prob = np.concatenate([o["oprob"] for o in outs], axis=0)
    inten = np.concatenate([o["oint"] for o in outs], axis=0)
    idx = np.concatenate([o["oidx"] for o in outs], axis=0).astype(np.int32)
    return (x_aug, prob, inten, idx), res


def kernel(**inputs):
    out, _ = run_spmd(inputs)
    return out
